# revision 20
# baseline (speedup 1.0000x reference)
"""Trainium2 Bass kernel for nn_Attention (linear attention w/ cubed feature map).

Math (per batch b):
  q = relu(in1 @ W.T + pos) / s ;  k = relu(in2 + pos) / s ;  s = softplus(scale_p)
  qf = (||q||/||q^3||) * q^3    ;  kf = (||k||/||k^3||) * k^3
  kv[h] = (1/N) * kf_h.T @ v_h  (v = in2),  per head h (32-dim blocks)
  out = sigmoid(q_f @ blockdiag(kv)) * in1

Distribution: sequence-parallel over N across 8 cores; per-(b,h) kv partials
are AllReduce'd (tiny), everything else local.

Device layouts: k-side rows-major [rows=128part, feat]; q-side feature-major
[feat=128part, rows] (input1/pos/output are transposed HOST-side so no device
transposes are needed anywhere).
"""

import numpy as np

B, N, D, H = 4, 16384, 256, 8
NCORES = 8
NS = N // NCORES          # 2048 positions per core
PK = 4                    # k-side row-tiles per pack
NPK = (NS // 128) // PK   # packs per batch = 4
ST = 512                  # q-side supertile rows
NST = NS // ST            # supertiles per batch = 4

# dtype config: "f32" everywhere is the safe baseline; "bf16" accelerates
# DVE tensor_tensor (2x) and matmuls. Set via build(cfg).
DEFAULT_CFG = dict(
    bf16_k=True,     # k-side mid tensors (u, t', k2, k3, vp) in bf16
    bf16_q=False,    # q-side mid tensors (u', q2, q3, w2, wq) in bf16
    proj_f32=True,   # projection matmul in exact fp32 (accuracy)
    sim=False,       # single-core variant w/o collective (TimelineSim only)
    psq_bufs=2, px_bufs=1, qa_bufs=6, qb_bufs=3,
)

_BUILT = {}


def _mmdt(mybir, ap, bf16):
    """matmul operand dtype view: bf16 tiles pass through, f32 -> float32r."""
    if ap.dtype == mybir.dt.float32:
        return ap.bitcast(mybir.dt.float32r)
    return ap


def build(cfg=None):
    cfg = dict(DEFAULT_CFG, **(cfg or {}))
    key = tuple(sorted(cfg.items()))  # includes any extra knobs
    if key in _BUILT:
        return _BUILT[key]

    import concourse.bacc as bacc
    import concourse.mybir as mybir
    import concourse.tile as tile

    f32 = mybir.dt.float32
    f32r = mybir.dt.float32r
    bf16 = mybir.dt.bfloat16
    kdt = bf16 if cfg["bf16_k"] else f32
    qdt = bf16 if cfg["bf16_q"] else f32
    kmm = bf16 if cfg["bf16_k"] else f32r   # k-side matmul operands
    qmm = bf16 if cfg["bf16_q"] else f32r   # q-side matmul operands

    def asf32(ap):
        return ap.bitcast(f32) if ap.dtype == f32r else ap
    AF = mybir.ActivationFunctionType
    ALU = mybir.AluOpType

    nc = bacc.Bacc("TRN2", target_bir_lowering=False, debug=False,
                   num_devices=(1 if cfg["sim"] else NCORES))

    in2_d = nc.dram_tensor("in2", [B, NS, D], kdt, kind="ExternalInput")
    in1t_d = nc.dram_tensor("in1t", [B, D, NS], f32r, kind="ExternalInput")
    posp_d = nc.dram_tensor("posp", [NS, D], kdt, kind="ExternalInput")
    post_d = nc.dram_tensor("post", [D, NS], f32, kind="ExternalInput")
    wt_d = nc.dram_tensor("wt", [D, D], f32r, kind="ExternalInput")
    isb_d = nc.dram_tensor("isb", [128, PK, D], kdt, kind="ExternalInput")
    sel_d = nc.dram_tensor("sel", [128, NST * 4], qmm, kind="ExternalInput")
    selr_d = nc.dram_tensor("selr", [4, NST * 128], f32r, kind="ExternalInput")
    mask_d = nc.dram_tensor("mask", [2, 128, D], f32, kind="ExternalInput")
    outt_d = nc.dram_tensor("outt", [B, D, NS], f32, kind="ExternalOutput")

    in2_r = in2_d.ap().rearrange("b (pk t p) f -> b pk p t f", pk=NPK, t=PK, p=128)
    posp_r = posp_d.ap().rearrange("(T p) f -> p T f", p=128)
    post_r = post_d.ap().rearrange("(c p) r -> c p r", p=128)
    in1t_r = in1t_d.ap().rearrange("b (c p) r -> b c p r", p=128)
    wt_r = wt_d.ap().rearrange("(c p) e -> c p e", p=128)
    mask_r = mask_d.ap().rearrange("c p f -> p c f")
    outt_r = outt_d.ap().rearrange("b (c p) r -> b c p r", p=128)

    with tile.TileContext(nc) as tc:
        with (
            tc.tile_pool(name="const", bufs=1) as constp,
            tc.tile_pool(name="dram", bufs=1, space="DRAM") as dram,
            tc.tile_pool(name="ka", bufs=cfg.get("ka_bufs", 2)) as kap,
            tc.tile_pool(name="kb", bufs=2) as kbp,
            tc.tile_pool(name="ksml", bufs=3) as ksml,
            tc.tile_pool(name="qa", bufs=cfg.get("qa_bufs", 5)) as qap,
            tc.tile_pool(name="qb", bufs=cfg.get("qb_bufs", 2)) as qbp,
            tc.tile_pool(name="qs", bufs=1) as qsp,
            tc.tile_pool(name="kvps", bufs=2, space="PSUM") as kvpsp,
            tc.tile_pool(name="psq", bufs=cfg.get("psq_bufs", 1), space="PSUM") as psqp,
            tc.tile_pool(name="pss", bufs=1, space="PSUM") as pssp,
            tc.tile_pool(name="prb", bufs=1, space="PSUM") as prbp,
            tc.tile_pool(name="px", bufs=cfg.get("px_bufs", 2), space="PSUM") as pxp,
        ):
            # ---- resident constants ----
            isb_sb = constp.tile([128, PK, D], kdt, tag="isb")
            nc.sync.dma_start(out=isb_sb[:], in_=isb_d.ap())
            post_sb = constp.tile([128, 2, NS], f32, tag="post")
            wt_sb = constp.tile([128, 2, D], f32r, tag="wt")
            sel_sb = constp.tile([128, NST * 4], qmm, tag="sel")
            selr_sb = constp.tile([4, NST * 128], f32r, tag="selr")
            mask_sb = constp.tile([128, 2, D], f32, tag="mask")

            def load_phase_b_consts():
                for c in range(2):
                    nc.sync.dma_start(out=post_sb[:, c, :], in_=post_r[c])
                for c in range(2):
                    nc.sync.dma_start(out=wt_sb[:, c, :], in_=wt_r[c])
                nc.sync.dma_start(out=sel_sb[:], in_=sel_d.ap())
                nc.sync.dma_start(out=selr_sb[:], in_=selr_d.ap())
                nc.sync.dma_start(out=mask_sb[:], in_=mask_r)

            cc_in = dram.tile([B, 128, 2, D], f32)
            cc_out = dram.tile([B, 128, 2, D], f32)
            cc_done = []

            # ================= PHASE A: k-side, rows-major =================
            # two half-passes (b in {0,1}, then {2,3}) so only 2 kv psum
            # banks are live at a time; posp is re-streamed per half.
            kv_ps = {}
            for half in range(2):
              bs = (2 * half, 2 * half + 1)
              for b in bs:
                kv_ps[b] = kvpsp.tile([128, 2, D], f32, tag="kv", name=f"kvps{b}")
              for pk in range(NPK):
                posp_t = kap.tile([128, PK, D], kdt, tag="posp",
                                  name=f"posp_{half}_{pk}")
                nc.sync.dma_start(
                    out=posp_t[:], in_=posp_r[:, pk * PK:(pk + 1) * PK, :])
                for b in bs:
                    in2_t = kap.tile([128, PK, D], kdt, tag="in2")
                    nc.sync.dma_start(out=in2_t[:], in_=in2_r[b, pk])
                    # u = in2 * inv_s ; t' = u + posp  (fully scaled pre-act)
                    u = kap.tile([128, PK, D], kdt, tag="u")
                    nc.vector.tensor_mul(u[:], in2_t[:], isb_sb[:])
                    t_ = kap.tile([128, PK, D], kdt, tag="t_")
                    nc.gpsimd.tensor_add(t_[:], u[:], posp_t[:])
                    # k2 = relu(t')^2 (= relu(t')*t'), s1 = row-sums
                    k2 = kap.tile([128, PK, D], kdt, tag="k2")
                    s1c = ksml.tile([128, PK], f32, tag="s1c")
                    for t in range(PK):
                        nc.vector.scalar_tensor_tensor(
                            out=k2[:, t, :], in0=t_[:, t, :], scalar=0.0,
                            in1=t_[:, t, :], op0=ALU.max, op1=ALU.mult,
                            accum_out=s1c[:, t:t + 1])
                    # k3 = relu(t') * k2
                    k3 = kap.tile([128, PK, D], kmm, tag="k3")
                    nc.vector.tensor_mul(k3[:], k2[:], t_[:])
                    # s3 = row-sums of k3^2 (ACT engine, fused accum)
                    k6 = kbp.tile([128, PK, D], kdt, tag="k6")
                    s3c = ksml.tile([128, PK], f32, tag="s3c")
                    for t in range(PK):
                        nc.scalar.activation(
                            k6[:, t, :], asf32(k3[:, t, :]), AF.Square,
                            accum_out=s3c[:, t:t + 1])
                    # ratio_k = sqrt(s1/s3); v' = in2 * ratio_k
                    rec = ksml.tile([128, PK], f32, tag="rec")
                    nc.vector.reciprocal(rec[:], s3c[:])
                    rr = ksml.tile([128, PK], f32, tag="rr")
                    nc.vector.tensor_mul(rr[:], s1c[:], rec[:])
                    rat = ksml.tile([128, PK], f32, tag="rat")
                    nc.scalar.activation(rat[:], rr[:], AF.Sqrt)
                    vp = kbp.tile([128, PK, D], kmm, tag="vp")
                    for t in range(PK):
                        nc.vector.tensor_scalar_mul(
                            vp[:, t, :], in2_t[:, t, :], rat[:, t:t + 1])
                    # kv += k3.T @ v'  (full 256x256; block-diag masked later)
                    for t in range(PK):
                        for c in range(2):
                            nc.tensor.matmul(
                                kv_ps[b][:, c, :],
                                lhsT=_mmdt(mybir, k3[:, t, c * 128:(c + 1) * 128], cfg["bf16_k"]),
                                rhs=_mmdt(mybir, vp[:, t, :], cfg["bf16_k"]),
                                start=(pk == 0 and t == 0),
                                stop=(pk == NPK - 1 and t == PK - 1))
                if half == 0 and pk == 0 and b == 1:
                    load_phase_b_consts()
              # evacuate this half's kv psums -> sbuf -> DRAM bounce
              for b in bs:
                kv_sb = kbp.tile([128, 2, D], f32, tag="kvsb", name=f"kvsb{b}")
                nc.scalar.copy(kv_sb[:], kv_ps[b][:])
                nc.sync.dma_start(out=cc_in[b], in_=kv_sb[:])
              # AllReduce this half's kv partials (overlaps next half)
              if cfg["sim"]:
                  nc.sync.dma_start(out=cc_out[2 * half:2 * half + 2],
                                    in_=cc_in[2 * half:2 * half + 2])
              else:
                  nc.gpsimd.collective_compute(
                      "AllReduce", mybir.AluOpType.add,
                      replica_groups=[list(range(NCORES))],
                      ins=[cc_in[2 * half:2 * half + 2].opt()],
                      outs=[cc_out[2 * half:2 * half + 2].opt()])

            kvf = []
            for b in range(B):
                kvraw = constp.tile([128, 2, D], f32, tag=f"kvraw{b}")
                nc.sync.dma_start(out=kvraw[:], in_=cc_out[b])
                kvm = constp.tile([128, 2, D], qmm, tag=f"kvf{b}")
                nc.vector.tensor_mul(kvm[:], kvraw[:], mask_sb[:])
                kvf.append(kvm)

            # ================= PHASE B: q-side, feature-major ==============
            for b in range(B):
                ps14 = pssp.tile([4, ST], f32, tag="ps14")
                ps34 = pssp.tile([4, ST], f32, tag="ps34")
                a1s, q3s = [], []
                for st in range(NST):
                    a1 = [qap.tile([128, ST], f32r, tag=f"a1_{c}", name=f"a1_{b}_{st}_{c}") for c in range(2)]
                    for c in range(2):
                        nc.sync.dma_start(
                            out=a1[c][:],
                            in_=in1t_r[b, c, :, st * ST:(st + 1) * ST])
                    a1s.append(a1)
                    q3j = []
                    pf32 = cfg.get("proj_f32", False)
                    for j in range(2):
                        psq = psqp.tile([128, ST], f32, tag="psq")
                        for c in range(2):
                            lh = wt_sb[:, c, j * 128:(j + 1) * 128]
                            rh = a1[c][:]
                            if pf32:
                                lh = lh.bitcast(f32)
                                rh = rh.bitcast(f32)
                            nc.tensor.matmul(
                                psq[:], lhsT=lh, rhs=rh,
                                start=(c == 0), stop=(c == 1))
                        up = qbp.tile([128, ST], qdt, tag="up")
                        nc.vector.tensor_add(
                            up[:], psq[:],
                            post_sb[:, j, st * ST:(st + 1) * ST])
                        q2 = qbp.tile([128, ST], qmm, tag="q2")
                        nc.vector.scalar_tensor_tensor(
                            out=q2[:], in0=up[:], scalar=0.0, in1=up[:],
                            op0=ALU.max, op1=ALU.mult)
                        q3 = qap.tile([128, ST], qdt, tag=f"q3_{j}")
                        nc.vector.tensor_mul(q3[:], asf32(q2[:]), up[:])
                        q3j.append(q3)
                        w2 = qbp.tile([128, ST], qmm, tag="w2")
                        nc.scalar.activation(w2[:], q3[:], AF.Square)
                        nc.tensor.matmul(
                            ps14[:],
                            lhsT=_mmdt(mybir, sel_sb[:, st * 4:(st + 1) * 4], cfg["bf16_q"]),
                            rhs=q2[:],
                            start=(st == 0 and j == 0),
                            stop=(st == NST - 1 and j == 1))
                        nc.tensor.matmul(
                            ps34[:],
                            lhsT=_mmdt(mybir, sel_sb[:, st * 4:(st + 1) * 4], cfg["bf16_q"]),
                            rhs=w2[:],
                            start=(st == 0 and j == 0),
                            stop=(st == NST - 1 and j == 1))
                    q3s.append(q3j)
                # ratio_q = sqrt(s1/s3)/N for the whole batch b
                rec4 = qsp.tile([4, ST], f32, tag="rec4")
                nc.vector.reciprocal(rec4[:], ps34[:])
                rr4 = qsp.tile([4, ST], f32, tag="rr4")
                nc.vector.tensor_mul(rr4[:], ps14[:], rec4[:])
                rat4 = qsp.tile([4, ST], f32r, tag="rat4")
                nc.scalar.activation(rat4[:], rr4[:], AF.Sqrt,
                                     scale=1.0 / float(N) ** 2)
                for st in range(NST):
                    prb = prbp.tile([128, ST], f32, tag="prb")
                    nc.tensor.matmul(
                        prb[:],
                        lhsT=_mmdt(mybir, selr_sb[:, st * 128:(st + 1) * 128], False),
                        rhs=rat4[:],
                        start=True, stop=True)
                    if cfg["bf16_q"]:
                        rbs = qbp.tile([128, ST], qdt, tag="rbs")
                        nc.scalar.copy(rbs[:], prb[:])
                        rbsrc = rbs
                    else:
                        rbsrc = prb
                    wq = []
                    for j in range(2):
                        w = qbp.tile([128, ST], qmm, tag=f"wq_{j}")
                        nc.vector.tensor_mul(w[:], q3s[st][j][:], rbsrc[:])
                        wq.append(w)
                    for j in range(2):
                        px = pxp.tile([128, ST], f32, tag="px")
                        for c in range(2):
                            nc.tensor.matmul(
                                px[:],
                                lhsT=_mmdt(mybir, kvf[b][:, c, j * 128:(j + 1) * 128], cfg["bf16_q"]),
                                rhs=wq[c][:],
                                start=(c == 0), stop=(c == 1))
                        sg = qbp.tile([128, ST], qdt, tag="sg")
                        nc.scalar.activation(sg[:], px[:], AF.Sigmoid)
                        ot = qbp.tile([128, ST], f32, tag="ot")
                        nc.gpsimd.tensor_mul(ot[:], sg[:], asf32(a1s[st][j][:]))
                        nc.sync.dma_start(
                            out=outt_r[b, j, :, st * ST:(st + 1) * ST],
                            in_=ot[:])

    nc.compile()
    _BUILT[key] = nc
    return nc


def _prep_inputs(input1, input2, conv_w, pos_enc, scale_p, cfg=None):
    import ml_dtypes
    cfg = dict(DEFAULT_CFG, **(cfg or {}))
    kdt = ml_dtypes.bfloat16 if cfg["bf16_k"] else np.float32
    qdt = ml_dtypes.bfloat16 if cfg["bf16_q"] else np.float32

    inv_s = (1.0 / np.logaddexp(scale_p.reshape(-1).astype(np.float64), 0.0)
             ).astype(np.float32)                      # [256]
    wtp = np.ascontiguousarray(
        (conv_w.astype(np.float64) * inv_s[:, None].astype(np.float64)
         ).T.astype(np.float32))                       # [d, e] lhsT layout
    posp_full = (pos_enc[0].astype(np.float64)
                 * inv_s[None, :].astype(np.float64)).astype(np.float32)
    in1t_full = np.ascontiguousarray(input1.transpose(0, 2, 1))

    isb = np.ascontiguousarray(
        np.broadcast_to(inv_s, (128, PK, D))).astype(kdt)
    sel = np.zeros((128, NST * 4), dtype=qdt)
    for st in range(NST):
        sel[:, st * 4 + st] = 1
    selr = np.zeros((4, NST * 128), dtype=np.float32)
    for st in range(NST):
        selr[st, st * 128:(st + 1) * 128] = 1
    heads = np.arange(D) // (D // H)
    mask2 = (heads[:, None] == heads[None, :]).astype(np.float32)  # [256,256]
    mask = np.ascontiguousarray(mask2.reshape(2, 128, D))

    in_maps = []
    for core in range(NCORES):
        sl = slice(core * NS, (core + 1) * NS)
        in_maps.append({
            "in2": np.ascontiguousarray(input2[:, sl, :]).astype(kdt),
            "in1t": np.ascontiguousarray(in1t_full[:, :, sl]),
            "posp": np.ascontiguousarray(posp_full[sl]).astype(kdt),
            "post": np.ascontiguousarray(posp_full[sl].T),
            "wt": wtp,
            "isb": isb,
            "sel": sel,
            "selr": selr,
            "mask": mask,
        })
    return in_maps


def kernel(input1, input2, conv_w, pos_enc, scale_p, _cfg=None, _trace=False):
    from concourse import bass_utils
    nc = build(_cfg)
    in_maps = _prep_inputs(input1, input2, conv_w, pos_enc, scale_p, _cfg)
    res = bass_utils.run_bass_kernel_spmd(
        nc, in_maps, core_ids=list(range(NCORES)), trace=_trace)
    out = np.concatenate(
        [r["outt"].transpose(0, 2, 1) for r in res.results], axis=1)
    kernel._last_results = res
    return np.ascontiguousarray(out.astype(np.float32))
